# revision 14
# baseline (speedup 1.0000x reference)
"""Multi-head differential attention Trainium2 kernel (8 NeuronCores).

Sharding: core c -> batch b = c // 4, head group g = c % 4 (4 of 16 heads).
Each core computes its heads' projections, attention, per-head layernorm and
its partial slice of the output projection; the host sums the 4 partials per
batch (standard tensor-parallel unshard) and adds the output bias.

Math notes:
 - The reference masks whole query rows with -1e9 then softmaxes, which gives
   uniform attention on masked rows.  We zero the (scaled) q rows instead:
   scores become 0, exp=1, softmax uniform -- identical result.
 - Layernorm is invariant to positive per-row scaling, so instead of
   normalizing the two softmaxes we feed LN with
       y'' = r2 * y1 - (lam * r1) * y2  (= r1*r2 * (y1/r1 - lam*y2/r2))
   where r1/r2 are the exp-row-sums.  No reciprocals needed anywhere.
 - The trailing (1 - lambda_init) factor is folded into ln_w / ln_b.
"""

import math
import sys

sys.path.insert(0, "/opt/trn_rl_repo")

import ml_dtypes
import numpy as np

import concourse.bass as bass
import concourse.bass_isa as bass_isa
import concourse.mybir as mybir
from concourse import bacc
from concourse.bass import ds, ts
from concourse.bass_utils import run_bass_kernel_spmd
from concourse.tile import TileContext

B, T, C, H = 2, 2048, 1024, 16
HS = C // H            # 64
D2 = 2 * HS            # 128
LAYER_IDX = 2
LAMBDA_INIT = 0.8 - 0.6 * float(np.exp(-0.3 * (LAYER_IDX - 1)))
EPS = 1e-9
N_CORES = 8
HPC = H // (N_CORES // B)   # heads per core = 4

FP32 = mybir.dt.float32
BF16 = mybir.dt.bfloat16
AF = mybir.ActivationFunctionType
ALU = mybir.AluOpType

_CACHED = {}


def build_nc(repeat=1, mode='all'):
    nc = bacc.Bacc("TRN2", target_bir_lowering=False, debug=False,
                   enable_asserts=False)

    xq_d = nc.dram_tensor("xq", [T, C], BF16, kind="ExternalInput").ap()
    xk_d = nc.dram_tensor("xk", [T, C], BF16, kind="ExternalInput").ap()
    xv_d = nc.dram_tensor("xv", [T, C], BF16, kind="ExternalInput").ap()
    # mask row (float), NOT pre-scaled
    mask_d = nc.dram_tensor("maskf", [1, T], FP32, kind="ExternalInput").ap()
    # weights, host packed to SBUF layout (partition dim first)
    wq_d = nc.dram_tensor("wq", [128, HPC * 8 * 128], BF16, kind="ExternalInput").ap()
    wk_d = nc.dram_tensor("wk", [128, HPC * 8 * 128], BF16, kind="ExternalInput").ap()
    wv_d = nc.dram_tensor("wv", [128, 8 * 512], BF16, kind="ExternalInput").ap()
    wc_d = nc.dram_tensor("wc", [128, HPC * 1024], BF16, kind="ExternalInput").ap()
    lnw_d = nc.dram_tensor("lnw", [128, 1], FP32, kind="ExternalInput").ap()
    lnb_d = nc.dram_tensor("lnb", [128, 1], FP32, kind="ExternalInput").ap()
    lq1_d = nc.dram_tensor("lq1", [1, HPC * HS], FP32, kind="ExternalInput").ap()
    lk1_d = nc.dram_tensor("lk1", [1, HPC * HS], FP32, kind="ExternalInput").ap()
    lq2_d = nc.dram_tensor("lq2", [1, HPC * HS], FP32, kind="ExternalInput").ap()
    lk2_d = nc.dram_tensor("lk2", [1, HPC * HS], FP32, kind="ExternalInput").ap()
    out_d = nc.dram_tensor("out", [T, C], FP32, kind="ExternalOutput").ap()

    NQS = 4            # q slices of 512
    QS = T // NQS
    NKT = T // 128     # 16 k tiles

    with TileContext(nc) as tc:
      for _rep in range(repeat):
        with (
            tc.tile_pool(name="singles", bufs=1) as singles,
            tc.tile_pool(name="proj", bufs=1) as proj_pool,
        ):
            # ---------- constants / tiny prep ----------
            lnw_sb = singles.tile([128, 1], FP32, tag="lnw")
            lnb_sb = singles.tile([128, 1], FP32, tag="lnb")
            nc.sync.dma_start(out=lnw_sb, in_=lnw_d)
            nc.sync.dma_start(out=lnb_sb, in_=lnb_d)

            # lambda per head: lam = exp(sum(lq1*lk1)) - exp(sum(lq2*lk2)) + l0
            lrow = singles.tile([1, HPC * HS], FP32, tag="lrow")
            lrow2 = singles.tile([1, HPC * HS], FP32, tag="lrow2")
            ltmp = singles.tile([1, HPC * HS], FP32, tag="ltmp")
            s1 = singles.tile([1, HPC], FP32, tag="s1")
            s2 = singles.tile([1, HPC], FP32, tag="s2")
            lam_row = singles.tile([1, HPC], FP32, tag="lam_row")
            nc.sync.dma_start(out=lrow, in_=lq1_d)
            nc.sync.dma_start(out=lrow2, in_=lk1_d)
            nc.vector.tensor_mul(ltmp, lrow, lrow2)
            nc.vector.reduce_sum(s1, ltmp.rearrange("p (h d) -> p h d", d=HS),
                                 axis=mybir.AxisListType.X)
            nc.sync.dma_start(out=lrow, in_=lq2_d)
            nc.sync.dma_start(out=lrow2, in_=lk2_d)
            nc.vector.tensor_mul(ltmp, lrow, lrow2)
            nc.vector.reduce_sum(s2, ltmp.rearrange("p (h d) -> p h d", d=HS),
                                 axis=mybir.AxisListType.X)
            nc.scalar.activation(s1, s1, AF.Exp)
            nc.scalar.activation(s2, s2, AF.Exp)
            nc.vector.tensor_sub(lam_row, s1, s2)
            nc.vector.tensor_scalar_add(lam_row, lam_row, LAMBDA_INIT)
            lam_col = singles.tile([128, HPC], FP32, tag="lam_col")
            nc.gpsimd.partition_broadcast(lam_col, lam_row, 128)

            # mask row -> scale*mask broadcast to [128, T] (bf16)
            mrow = singles.tile([1, T], FP32, tag="mrow")
            mrow_b = singles.tile([1, T], BF16, tag="mrow_b")
            mb = singles.tile([128, T], BF16, tag="mb")
            nc.sync.dma_start(out=mrow, in_=mask_d)
            nc.vector.tensor_scalar(mrow_b, mrow, 1.0 / math.sqrt(HS), None,
                                    op0=ALU.mult)
            nc.gpsimd.partition_broadcast(mb, mrow_b, 128)

            wc_sb = singles.tile([128, HPC * 1024], BF16, tag="wc")
            nc.sync.dma_start(out=wc_sb, in_=wc_d)

            # ---------- phase P2: transposed x loads + projections ----------
            qmapT = [proj_pool.tile([128, T], BF16, tag=f"qm{h}", name=f"qm{h}")
                     for h in range(HPC)]
            kmapT = [proj_pool.tile([128, T], BF16, tag=f"km{h}", name=f"km{h}")
                     for h in range(HPC)]
            vv = [proj_pool.tile([128, 4 * D2], BF16, tag=f"vv{i}", name=f"vv{i}")
                  for i in range(NKT)]

            with (
                tc.tile_pool(name="wpool", bufs=1) as wpool,
                tc.tile_pool(name="xt", bufs=12) as xt_pool,
                tc.tile_pool(name="ppsum", bufs=4, space="PSUM") as ppsum,
            ):
                wq_sb = wpool.tile([128, HPC * 8 * 128], BF16, tag="wq")
                wk_sb = wpool.tile([128, HPC * 8 * 128], BF16, tag="wk")
                wv_sb = wpool.tile([128, 8 * 512], BF16, tag="wv")
                nc.sync.dma_start(out=wq_sb, in_=wq_d)
                nc.sync.dma_start(out=wk_sb, in_=wk_d)
                nc.sync.dma_start(out=wv_sb, in_=wv_d)

                def w_qk(w_sb, h, ct):   # [128, 128] lhsT (C-tile ct, head h)
                    return w_sb[:, ds((h * 8 + ct) * 128, 128)]

                def load_xt(x_d, nm):
                    tiles = []
                    for i in range(8):
                        xt = xt_pool.tile([128, T], BF16, tag="xt",
                                          name=f"{nm}{i}")
                        nc.sync.dma_start_transpose(xt, x_d[:, ds(i * 128, 128)])
                        tiles.append(xt)
                    return tiles

                xqT = load_xt(xq_d, "xq")
                for h in range(HPC):
                    qt = qmapT[h]
                    for qs in range(NQS):
                        ps = ppsum.tile([128, QS], FP32, tag="ppsum")
                        for ct in range(8):
                            nc.tensor.matmul(ps, w_qk(wq_sb, h, ct),
                                             xqT[ct][:, ds(qs * QS, QS)],
                                             start=(ct == 0), stop=(ct == 7))
                        nc.vector.tensor_copy(qt[:, ds(qs * QS, QS)], ps)
                    nc.vector.tensor_mul(qt, qt, mb)   # mask+scale both maps

                xkT = load_xt(xk_d, "xk")
                for h in range(HPC):
                    kt_t = kmapT[h]
                    for qs in range(NQS):
                        ps = ppsum.tile([128, QS], FP32, tag="ppsum")
                        for ct in range(8):
                            nc.tensor.matmul(ps, w_qk(wk_sb, h, ct),
                                             xkT[ct][:, ds(qs * QS, QS)],
                                             start=(ct == 0), stop=(ct == 7))
                        nc.vector.tensor_copy(kt_t[:, ds(qs * QS, QS)], ps)

                xvT = load_xt(xv_d, "xv")
                for kt in range(NKT):
                    ps = ppsum.tile([128, 512], FP32, tag="ppsum")
                    for ct in range(8):
                        nc.tensor.matmul(ps, xvT[ct][:, ds(kt * 128, 128)],
                                         wv_sb[:, ds(ct * 512, 512)],
                                         start=(ct == 0), stop=(ct == 7))
                    nc.vector.tensor_copy(vv[kt], ps)

            # ---------- attention ----------
            eps_col = singles.tile([128, 1], FP32, tag="eps_col")
            nc.vector.memset(eps_col, EPS)
            ynormT = [proj_pool.tile([128, T], BF16, tag=f"yn{h}", name=f"yn{h}")
                      for h in range(HPC)]
            with (
                tc.tile_pool(name="escr", bufs=3) as e_pool,
                tc.tile_pool(name="scr", bufs=1) as scr_pool,
                tc.tile_pool(name="spsum", bufs=2, space="PSUM") as spsum,
                tc.tile_pool(name="ypsum", bufs=4, space="PSUM") as ypsum,
            ):
                for h in range(HPC):
                    vslice = ds(h * D2, D2)
                    # per-head buffers (epilogue batched across the 4 q-slices)
                    rbig = scr_pool.tile([128, 2 * T], BF16, tag="rbig")
                    y1h = scr_pool.tile([128, T], FP32, tag="y1h")
                    y2h = scr_pool.tile([128, T], FP32, tag="y2h")
                    for qs in range(NQS):
                        qsl = ds(qs * QS, QS)
                        y1 = ypsum.tile([128, QS], FP32, tag="y")
                        y2 = ypsum.tile([128, QS], FP32, tag="y")
                        ra0 = scr_pool.tile([128, 2 * QS], BF16, tag="ra0",
                                            bufs=2)
                        ra1 = scr_pool.tile([128, 2 * QS], BF16, tag="ra1",
                                            bufs=2)
                        for kt in range(NKT):
                            ksl = ds(kt * 128, 128)
                            s = spsum.tile([128, 2 * QS], FP32, tag="s")
                            nc.tensor.matmul(s[:, 0:QS],
                                             kmapT[h][0:64, ksl],
                                             qmapT[h][0:64, qsl],
                                             start=True, stop=True,
                                             tile_position=(0, 0))
                            nc.tensor.matmul(s[:, QS:2 * QS],
                                             kmapT[h][64:128, ksl],
                                             qmapT[h][64:128, qsl],
                                             start=True, stop=True,
                                             tile_position=(64, 0))
                            e = e_pool.tile([128, 2 * QS], BF16, tag="e")
                            nc.scalar.activation(e, s, AF.Exp)
                            nc.tensor.matmul(y1, vv[kt][:, vslice], e[:, 0:QS],
                                             start=(kt == 0), stop=(kt == NKT - 1))
                            nc.tensor.matmul(y2, vv[kt][:, vslice],
                                             e[:, QS:2 * QS],
                                             start=(kt == 0), stop=(kt == NKT - 1))
                            # exp-row-sum accumulation: two DVE chains
                            tgt = ra0 if kt % 2 == 0 else ra1
                            if kt < 2:
                                nc.vector.tensor_copy(tgt, e)
                            else:
                                nc.vector.tensor_add(tgt, tgt, e)
                        # [r1_qs | r2_qs] into the head-wide row-sum buffer
                        nc.vector.tensor_add(
                            rbig[:, ds(qs * 2 * QS, 2 * QS)], ra0, ra1)
                        nc.vector.tensor_copy(y1h[:, qsl], y1)
                        nc.vector.tensor_copy(y2h[:, qsl], y2)

                    # ---- per-head epilogue ----
                    rall = scr_pool.tile([128, 2 * T], FP32, tag="rall")
                    if mode == "no_pool":
                        nc.vector.tensor_copy(rall, rbig)
                    else:
                        nc.gpsimd.partition_all_reduce(rall, rbig, 128,
                                                       bass_isa.ReduceOp.add)
                    rall3 = rall.rearrange("p (qs two q) -> p qs two q", two=2,
                                           q=QS)
                    r1v = rall3[:, :, 0, :]
                    r2v = rall3[:, :, 1, :]
                    y1v = y1h.rearrange("p (qs q) -> p qs q", q=QS)
                    y2v = y2h.rearrange("p (qs q) -> p qs q", q=QS)
                    # y'' = r2*y1 - (lam*r1)*y2  (LN is scale-invariant)
                    c2 = scr_pool.tile([128, T], FP32, tag="c2")
                    c2v = c2.rearrange("p (qs q) -> p qs q", q=QS)
                    nc.vector.tensor_scalar(c2v, r1v, lam_col[:, ds(h, 1)],
                                            None, op0=ALU.mult)
                    sln = scr_pool.tile([128, 2 * T], BF16, tag="sln")
                    yln = sln[:, 0:T]
                    ysq = sln[:, T:2 * T]
                    nc.vector.tensor_mul(y1v, y1v, r2v)
                    nc.vector.tensor_mul(c2v, y2v, c2v)
                    nc.vector.tensor_sub(yln, y1h, c2)
                    nc.vector.tensor_mul(ysq, yln, yln)
                    sred = scr_pool.tile([128, 2 * T], FP32, tag="sred")
                    if mode == "no_pool":
                        nc.vector.tensor_copy(sred, sln)
                    else:
                        nc.gpsimd.partition_all_reduce(sred, sln, 128,
                                                       bass_isa.ReduceOp.add)
                    mean = scr_pool.tile([128, T], FP32, tag="mean")
                    var = scr_pool.tile([128, T], FP32, tag="var")
                    nc.vector.tensor_scalar(mean, sred[:, 0:T], 1.0 / D2, None,
                                            op0=ALU.mult)
                    nc.vector.tensor_scalar(var, sred[:, T:2 * T], 1.0 / D2,
                                            None, op0=ALU.mult)
                    msq = scr_pool.tile([128, T], FP32, tag="c2", name="msq")
                    nc.vector.tensor_mul(msq, mean, mean)
                    nc.vector.tensor_sub(var, var, msq)
                    # rstd = exp(-0.5 * ln(var + eps))
                    nc.scalar.activation(var, var, AF.Ln, bias=eps_col)
                    nc.scalar.activation(var, var, AF.Exp, scale=-0.5)
                    nc.vector.tensor_sub(yln, yln, mean)
                    nc.vector.tensor_mul(yln, yln, var)
                    nc.vector.tensor_scalar(ynormT[h], yln, lnw_sb, lnb_sb,
                                            op0=ALU.mult, op1=ALU.add)

            # ---------- output projection ----------
            with (
                tc.tile_pool(name="obuf", bufs=2) as ob_pool,
                tc.tile_pool(name="opsum", bufs=4, space="PSUM") as opsum,
            ):
                for qt_i in range(T // 128):
                    qsl = ds(qt_i * 128, 128)
                    ob = ob_pool.tile([128, C], FP32, tag="ob")
                    for cs in range(2):
                        ps = opsum.tile([128, 512], FP32, tag="op")
                        for h in range(HPC):
                            nc.tensor.matmul(ps, ynormT[h][:, qsl],
                                             wc_sb[:, ds(h * 1024 + cs * 512, 512)],
                                             start=(h == 0), stop=(h == HPC - 1))
                        nc.vector.tensor_copy(ob[:, ds(cs * 512, 512)], ps)
                    nc.sync.dma_start(out=out_d[qsl, :], in_=ob)

    # Force every activation (Exp + Ln) onto the combined
    # natural_log_exp_and_others table set so the epilogue's Ln/Exp pair
    # doesn't thrash ACT_TABLE_LOADs against the attention Exps (~2.7us per
    # switch, 32 switches otherwise).  Emptying the other sets keeps the
    # set-id numbering (index into act_info.json) intact.
    _orig_tables = bacc.get_activation_tables

    def _only_combined(arch):
        out = {}
        for name, funcs in _orig_tables(arch).items():
            out[name] = funcs if name == "natural_log_exp_and_others" else set()
        return out

    bacc.get_activation_tables = _only_combined
    try:
        nc.compile()
    finally:
        bacc.get_activation_tables = _orig_tables
    return nc


def _prep_core_inputs(inputs, core):
    b = core // (N_CORES // B)
    g = core % (N_CORES // B)
    hs = slice(g * HPC * HS, (g + 1) * HPC * HS)          # head-dim cols (64/head)
    h2 = slice(g * HPC * D2, (g + 1) * HPC * D2)          # 128/head cols
    bf = ml_dtypes.bfloat16

    def pack_qk(w1, w2):
        # -> [128, HPC*8*128]: per head the 8 C-tiles of [Wq1_h | Wq2_h]
        cols = []
        for h in range(HPC):
            hh = slice((g * HPC + h) * HS, (g * HPC + h + 1) * HS)
            w = np.concatenate([w1[:, hh], w2[:, hh]], axis=1)   # [1024, 128]
            cols.append(w.reshape(8, 128, 128))
        arr = np.stack(cols, 0)                    # [HPC, 8, 128, 128]
        return np.ascontiguousarray(
            arr.transpose(2, 0, 1, 3).reshape(128, -1)).astype(bf)

    wv = inputs["Wv"][:, h2].reshape(8, 128, HPC * D2)
    wv = np.ascontiguousarray(wv.transpose(1, 0, 2).reshape(128, -1)).astype(bf)
    wc = inputs["Wc"][h2, :].reshape(HPC, 128, C)
    wc = np.ascontiguousarray(wc.transpose(1, 0, 2).reshape(128, -1)).astype(bf)

    sc = np.float32(1.0 - LAMBDA_INIT)
    heads = slice(g * HPC, (g + 1) * HPC)
    return {
        "xq": inputs["q"][b].astype(bf),
        "xk": inputs["k"][b].astype(bf),
        "xv": inputs["v"][b].astype(bf),
        "maskf": inputs["mask"][b].astype(np.float32).reshape(1, T),
        "wq": pack_qk(inputs["Wq1"], inputs["Wq2"]),
        "wk": pack_qk(inputs["Wk1"], inputs["Wk2"]),
        "wv": wv,
        "wc": wc,
        "lnw": (inputs["ln_w"] * sc).astype(np.float32).reshape(128, 1),
        "lnb": (inputs["ln_b"] * sc).astype(np.float32).reshape(128, 1),
        "lq1": inputs["lq1"][heads].astype(np.float32).reshape(1, -1),
        "lk1": inputs["lk1"][heads].astype(np.float32).reshape(1, -1),
        "lq2": inputs["lq2"][heads].astype(np.float32).reshape(1, -1),
        "lk2": inputs["lk2"][heads].astype(np.float32).reshape(1, -1),
    }


def kernel(q, k, v, mask, Wq1, bq1, Wq2, bq2, Wk1, bk1, Wk2, bk2,
           Wv, bv, Wc, bc, ln_w, ln_b, lq1, lk1, lq2, lk2, **run_kw):
    inputs = dict(q=np.asarray(q), k=np.asarray(k), v=np.asarray(v),
                  mask=np.asarray(mask), Wq1=np.asarray(Wq1),
                  Wq2=np.asarray(Wq2), Wk1=np.asarray(Wk1), Wk2=np.asarray(Wk2),
                  Wv=np.asarray(Wv), Wc=np.asarray(Wc),
                  ln_w=np.asarray(ln_w), ln_b=np.asarray(ln_b),
                  lq1=np.asarray(lq1), lk1=np.asarray(lk1),
                  lq2=np.asarray(lq2), lk2=np.asarray(lk2))
    if "nc" not in _CACHED:
        _CACHED["nc"] = build_nc()
    nc = _CACHED["nc"]
    in_maps = [_prep_core_inputs(inputs, c) for c in range(N_CORES)]
    res = run_bass_kernel_spmd(nc, in_maps, list(range(N_CORES)), **run_kw)
    _CACHED["last_results"] = res
    gpb = N_CORES // B
    out = np.zeros((B, T, C), np.float32)
    for c in range(N_CORES):
        out[c // gpb] += res.results[c]["out"]
    out += np.asarray(bc, np.float32)[None, None, :]
    return out
